# revision 13
# baseline (speedup 1.0000x reference)
"""Supervised-contrastive loss on 8 Trainium2 NeuronCores.

Math (reference):
    z = x / max(||x||, 1e-8)                  row-normalize
    sim = (z @ z.T) / TEMP                    [N, N]
    per-anchor: pos-mean over same-class (excl. self) and logsumexp over
    j != i, then per-class mean, then mean over classes.

exp(sim) is symmetric, so only half the matrix is computed ("wrapped
diagonal band"): anchors are split into 64 chunks of 128 rows; row-chunk
t computes column-chunks d = 0..32 ahead of it (mod 64).  A pair (i, j)
with chunk distance d is computed once (at the nearer row) for
1 <= d <= 31, at both rows for d == 32 -- the d=32 cell's exp carries
bias = -ln2 so each side contributes exactly half.  Row sums over the
band ride on the ScalarE Exp via accum_out; the "missing" transposed
halves are recovered as column sums: each exp tile (bf16, SBUF) is
added by the DVE into a per-core [128, 8192] accumulator, which is
DMA'd out raw and partition-reduced on the host.

Core c owns row-chunks t = c + 8k (k = 0..7).  Its z8 copy is
column-rotated by 128*c on the host so the band's SBUF addresses are
identical on every core (SPMD shares one instruction stream).  Class-
segment sums come from a small GEMM tm = A @ W.T with W[c] = sum of
z8 rows of class c (host-precomputed), so no masking is needed.  The
diagonal sim[i,i] = ||z8[i]||^2 is reconstructed exactly on host and
subtracted there.

Layout: all fp8 operands are host-packed for DoubleRow so that feature
d = kk*256 + i*128 + p lands on partition p, plane i of contraction tile
kk, giving 2KB-contiguous per-partition DMA descriptors.

Hardware notes baked into this structure: DMAs only from nc.sync,
one matmul accumulation group per PSUM bank, fp8 DoubleRow streams
1 output element per cycle per 256-deep pass (157 TF/s peak), ScalarE
is 1 elem/lane/cycle at 1.2 GHz (the old full-matrix kernel was
bottlenecked by it), and the d=32 runt cells are deferred to a tail
phase so the two rotating [128, 2048] PSUM slots never stall the PE
inside the main 8-row loop.
"""

import math

import numpy as np
import ml_dtypes

N = 8192           # anchors
D = 768            # feature dim
NOP = 64           # number of classes
CORES = 8
KT8 = D // 256     # 3 double-row contraction tiles
NROW = 8           # 128-row chunks per core
CELLW = 2048       # wide cell width (one PSUM slot, 4 banks)
RUNTW = 128        # d=32 runt cell width
BANDW = 33 * 128   # 4224 cols per row-chunk (d = 0..32)
GW = 2048          # z8 DMA group width
NG = N // GW       # 4 groups
TEMP_INV = 10.0
EPS = 1e-8

FP8 = ml_dtypes.float8_e4m3
BF16 = ml_dtypes.bfloat16

_CACHE = {}
LAST_RESULT = None  # BassKernelResults of the most recent run (for profiling)


def _splits(start, width):
    """Split a rotated-coords col range into <=2 non-wrapping pieces."""
    start %= N
    if start + width <= N:
        return [(start, width)]
    return [(start, N - start), (0, start + width - N)]


# acc col regions ready for DMA after runt k (runt k touches slice
# (1024k+4096) % N; rows touch slices per the wrapped band coverage)
ACC_DMA_AFTER_RUNT = {1: (4096, 6144), 3: (6144, 8192), 5: (0, 2048), 7: (2048, 4096)}


def _build_nc():
    from concourse import bacc
    import concourse.mybir as mybir
    import concourse.tile as tile

    f8 = mybir.dt.float8e4
    f32 = mybir.dt.float32
    bf16 = mybir.dt.bfloat16
    Exp = mybir.ActivationFunctionType.Exp
    DR = mybir.MatmulPerfMode.DoubleRow

    nc = bacc.Bacc(
        "TRN2", target_bir_lowering=False, debug=False, enable_asserts=False
    )
    z8 = nc.dram_tensor("z8", [128, NG, KT8, 2, GW], f8, kind="ExternalInput").ap()
    a8 = nc.dram_tensor("a8", [128, KT8, 2, NROW * 128], f8, kind="ExternalInput").ap()
    w8 = nc.dram_tensor("w8", [128, KT8, 2, NOP], f8, kind="ExternalInput").ap()
    tm = nc.dram_tensor("tm", [128, NROW, NOP], f32, kind="ExternalOutput").ap()
    pacc = nc.dram_tensor("pacc", [128, NROW, 3], f32, kind="ExternalOutput").ap()
    acc_out = nc.dram_tensor("acc_out", [128, N], bf16, kind="ExternalOutput").ap()

    with tile.TileContext(nc) as tc:
        with (
            tc.tile_pool(name="zin", bufs=NG) as zin,
            tc.tile_pool(name="epool", bufs=3) as epool,
            tc.tile_pool(name="singles", bufs=1) as singles,
        ):
            # ---- input DMAs (small/early operands first) ----
            w8_sb = singles.tile([128, KT8, 2, NOP], f8)
            nc.sync.dma_start(out=w8_sb, in_=w8)
            a8_sb = singles.tile([128, KT8, 2, NROW * 128], f8)
            nc.sync.dma_start(
                out=a8_sb.rearrange("p a b c -> p (a b c)"),
                in_=a8.rearrange("p a b c -> p (a b c)"),
            )
            z8_sb = {}
            for g in range(NG):
                z8_t = zin.tile([128, KT8, 2, GW], f8, name="z8_t", tag="z8_t")
                nc.sync.dma_start(
                    out=z8_t.rearrange("p a b c -> p (a b c)"),
                    in_=z8[:, g].rearrange("p a b c -> p (a b c)"),
                )
                z8_sb[g] = z8_t

            # colsum accumulator, zeroed while DMAs stream in
            acc = singles.tile([128, N], bf16)
            nc.vector.memset(acc, 0.0)

            # bias = -ln2 for the d=32 runt cells (halves their exp)
            nln2 = singles.tile([128, 1], f32)
            nc.vector.memset(nln2, -math.log(2.0))

            pacc_sb = singles.tile([128, NROW, 3], f32)
            tm_sb = singles.tile([128, NROW, NOP], f32)

            ps_pool = tc.alloc_tile_pool(name="ps", bufs=2, space="PSUM")

            # ---- class-segment sums: tm[:, k, c] = A_k @ W.T ----
            for k in range(NROW):
                pst = ps_pool.tile([128, NOP], f32, name="ps_t", tag="ps_t")
                for kk in range(KT8):
                    nc.tensor.matmul(
                        pst,
                        a8_sb[:, kk, :, k * 128:(k + 1) * 128],
                        w8_sb[:, kk, :, :],
                        start=(kk == 0),
                        stop=(kk == KT8 - 1),
                        perf_mode=DR,
                    )
                nc.vector.tensor_copy(tm_sb[:, k, :], pst)
            nc.sync.dma_start(out=tm, in_=tm_sb)

            def do_cell(k, ci, start, w, bias):
                """One band cell: sim matmuls -> Exp(+rowsum) -> DVE colsum."""
                ps_t = ps_pool.tile([128, w], f32, name="ps_t", tag="ps_t")
                for kk in range(KT8):
                    lhsT = a8_sb[:, kk, :, k * 128:(k + 1) * 128]
                    for jj in range(0, w, 512):
                        sw = min(512, w - jj)
                        g, off = divmod((start + jj) % N, GW)
                        nc.tensor.matmul(
                            ps_t[:, jj:jj + sw],
                            lhsT,
                            z8_sb[g][:, kk, :, off:off + sw],
                            start=(kk == 0),
                            stop=(kk == KT8 - 1),
                            perf_mode=DR,
                        )
                e_t = epool.tile([128, w], bf16, name="e_t", tag="e_t")
                nc.scalar.activation(
                    out=e_t,
                    in_=ps_t,
                    func=Exp,
                    scale=TEMP_INV,
                    bias=bias,
                    accum_out=pacc_sb[:, k, ci:ci + 1],
                )
                # colsum: skip the d=0 (diagonal) chunk at the band head
                eoff = RUNTW if ci == 0 else 0
                for s0, sw in _splits(start + eoff, w - eoff):
                    e0 = (s0 - start) % N
                    nc.vector.tensor_add(
                        acc[:, s0:s0 + sw],
                        acc[:, s0:s0 + sw],
                        e_t[:, e0:e0 + sw],
                    )

            # ---- main band: two wide cells per row-chunk ----
            for k in range(NROW):
                do_cell(k, 0, 1024 * k, CELLW, 0.0)
                do_cell(k, 1, 1024 * k + CELLW, CELLW, 0.0)

            # ---- d=32 runt cells (halved via bias=-ln2), with the acc
            # out-DMAs interleaved as their col regions become final ----
            for k in range(NROW):
                do_cell(k, 2, 1024 * k + 4096, RUNTW, nln2)
                if k in ACC_DMA_AFTER_RUNT:
                    lo, hi = ACC_DMA_AFTER_RUNT[k]
                    nc.sync.dma_start(out=acc_out[:, lo:hi], in_=acc[:, lo:hi])
            ps_pool.release()

            nc.sync.dma_start(out=pacc, in_=pacc_sb)

    nc.compile()
    return nc


def _get_nc():
    if "nc" not in _CACHE:
        _CACHE["nc"] = _build_nc()
    return _CACHE["nc"]


def _pack_dr(mat_t):
    """[D, cols] -> [128, KT8, 2, cols] with d = kk*256 + i*128 + p."""
    d, cols = mat_t.shape
    return np.ascontiguousarray(
        mat_t.reshape(KT8, 2, 128, cols).transpose(2, 0, 1, 3)
    )


def kernel(x, op_ids, n_op):
    global LAST_RESULT
    from concourse.bass_utils import run_bass_kernel_spmd

    x = np.asarray(x, dtype=np.float32).reshape(-1, D)
    op_ids = np.asarray(op_ids).reshape(-1).astype(np.int64)
    n_op_i = int(np.asarray(n_op))

    # ---- host prep: normalize, quantize, class sums, diagonal ----
    norms = np.sqrt((x.astype(np.float64) ** 2).sum(axis=1))
    norms = np.maximum(norms, EPS).astype(np.float32)
    z = x / norms[:, None]

    z8 = z.astype(FP8)
    z8f = z8.astype(np.float32)

    onehot = np.zeros((N, NOP), np.float32)
    onehot[np.arange(N), op_ids] = 1.0
    W8 = (onehot.T @ z8f).astype(FP8)               # [NOP, D] fp8

    z8_packed = _pack_dr(np.ascontiguousarray(z8.T))          # [128,3,2,N]
    w8_packed = _pack_dr(np.ascontiguousarray(W8.T.astype(FP8)))
    ssq = (z8f.astype(np.float64) ** 2).sum(axis=1)  # = sim[i, i]

    in_maps = []
    for c in range(CORES):
        rows = np.concatenate(
            [np.arange(128 * (c + 8 * k), 128 * (c + 8 * k) + 128)
             for k in range(NROW)]
        )
        a8_c = np.ascontiguousarray(z8_packed[:, :, :, rows])
        zrot = np.roll(z8_packed, -128 * c, axis=3)
        z8_c = np.ascontiguousarray(
            zrot.reshape(128, KT8, 2, NG, GW).transpose(0, 3, 1, 2, 4)
        )
        in_maps.append({"z8": z8_c, "a8": a8_c, "w8": w8_packed})

    nc = _get_nc()
    res = run_bass_kernel_spmd(nc, in_maps, core_ids=list(range(CORES)))
    LAST_RESULT = res

    # ---- host post: assemble es = rowsums + colsums, finish loss ----
    es = np.zeros(N, np.float64)
    tm_full = np.zeros((N, NOP), np.float64)
    for c in range(CORES):
        r = res.results[c]
        pacc_c = r["pacc"].astype(np.float64)      # [128, NROW, 3]
        tm_c = r["tm"].astype(np.float64)          # [128, NROW, NOP]
        cs = r["acc_out"].astype(np.float64).sum(axis=0)  # [8192] rotated
        es += np.roll(cs, 128 * c)                 # unrotate
        for k in range(NROW):
            t = c + 8 * k
            rows = slice(128 * t, 128 * t + 128)
            es[rows] += pacc_c[:, k, :].sum(axis=1)
            tm_full[rows] = tm_c[:, k, :]

    lse = np.log(es - np.exp(TEMP_INV * ssq))
    pos_sum = TEMP_INV * (tm_full[np.arange(N), op_ids] - ssq)
    counts = np.bincount(op_ids, minlength=n_op_i).astype(np.float64)
    pos_cnt = counts[op_ids] - 1.0

    loss_i = np.where(pos_cnt > 0, -pos_sum / np.maximum(pos_cnt, 1.0) + lse, 0.0)
    cls_sum = np.bincount(op_ids, weights=loss_i, minlength=n_op_i)
    cls_loss = np.where(counts > 0, cls_sum / np.maximum(counts, 1.0), 0.0)
    return np.float32(cls_loss.mean())
